# revision 17
# baseline (speedup 1.0000x reference)
"""Multi-head attention (B=4, N=2048, E=512, H=8) on 8 Trainium2 cores.

Sharding: core c -> (batch b = c//2, head-group g = c%2 of 4 heads).
Each core computes q/k/v projections for its 4 heads, full attention,
and a partial output projection (its heads' slice of Wo contraction);
the host sums the two partials per batch (fp16 partials).

v3 (spread att@v; ACT-saturated pipeline):
  - unified stream over (head, kchunk): PE emits energy(s,kc) -> ACT exp
    -> 4-kc-lagged att@v accumulation steps for the same head, so att@v
    never forms a serial tail (the old tail was ~75us at half clock)
  - att@v accumulates into packed PSUM tiles ([128,455]x2 + [128,130],
    16 q-chunks x 65 cols each, bank-straddle-free) across all 16 kc
  - per-q-chunk denominator comes from the ones-column of v_aug as
    before; normalization (DVE) right after each head's last att@v step
  - head-pair transposes for the output projection run early (pair 01
    during head 2's stage); only pair 23 + out-proj + fp16 output DMA
    remain in the ~10us tail
"""

import sys

if "/opt/trn_rl_repo" not in sys.path:
    sys.path.insert(0, "/opt/trn_rl_repo")

import numpy as np

B, N, E, H, D = 4, 2048, 512, 8, 64
NH = 4                      # heads per core
NCHUNK = N // 128           # 16
ECHUNK = E // 128           # 4
SCALE = float(1.0 / np.sqrt(E))
N_CORES = 8
LAG = 4                     # att@v trails energy by LAG kchunks

ATT_POOL_BUFS = 34          # shared [128,2048] fp16 slots: 12 xT tiles + 2-head attT window

_built = None


def _build():
    global _built
    if _built is not None:
        return _built

    from contextlib import ExitStack

    import concourse.bass as bass  # noqa: F401
    import concourse.mybir as mybir
    import concourse.tile as tile
    from concourse import bacc

    DT = mybir.dt.float16
    F32 = mybir.dt.float32
    AF = mybir.ActivationFunctionType

    nc = bacc.Bacc(
        "TRN2",
        target_bir_lowering=False,
        debug=False,
        num_devices=N_CORES,
    )

    xqT = nc.dram_tensor("xqT", [E, N], DT, kind="ExternalInput").ap()
    xkT = nc.dram_tensor("xkT", [E, N], DT, kind="ExternalInput").ap()
    xvT = nc.dram_tensor("xvT", [E, N], DT, kind="ExternalInput").ap()
    wqtd = nc.dram_tensor("wqtd", [E, 256], DT, kind="ExternalInput").ap()
    wktd = nc.dram_tensor("wktd", [E, 256], DT, kind="ExternalInput").ap()
    wvt = nc.dram_tensor("wvt", [E, NH * D], DT, kind="ExternalInput").ap()
    wot = nc.dram_tensor("wot", [NH * D, E], DT, kind="ExternalInput").ap()
    iden = nc.dram_tensor("iden", [128, 128], DT, kind="ExternalInput").ap()
    out = nc.dram_tensor("out", [N, E], DT, kind="ExternalOutput").ap()

    with tile.TileContext(nc) as tc, ExitStack() as ctx:
        consts = ctx.enter_context(tc.tile_pool(name="consts", bufs=1))
        big = ctx.enter_context(tc.tile_pool(name="big", bufs=ATT_POOL_BUFS))
        qk = ctx.enter_context(tc.tile_pool(name="qk", bufs=1))
        vp = ctx.enter_context(tc.tile_pool(name="vp", bufs=1))
        oallp = ctx.enter_context(tc.tile_pool(name="oall", bufs=1))
        otp = ctx.enter_context(tc.tile_pool(name="ot", bufs=1))
        ostage = ctx.enter_context(tc.tile_pool(name="ostage", bufs=3))
        smallp = ctx.enter_context(tc.tile_pool(name="small", bufs=4))

        # PSUM (8 banks): energy f32 [128,1024] x2 bufs (4 banks),
        # att@v accumulators 3 single-buf pools (1 bank each), misc 1 bank.
        # NOTE: matmul start=True zeros the whole 2KB bank (zero region), so
        # each pav bank gets exactly one start (first region) and one stop
        # (last region) per accumulation pass.
        ps_energy = ctx.enter_context(tc.tile_pool(name="ps_energy", bufs=2, space="PSUM"))
        ps_pavA = ctx.enter_context(tc.tile_pool(name="ps_pavA", bufs=1, space="PSUM"))
        ps_pavB = ctx.enter_context(tc.tile_pool(name="ps_pavB", bufs=1, space="PSUM"))
        ps_pavC = ctx.enter_context(tc.tile_pool(name="ps_pavC", bufs=1, space="PSUM"))
        ps_misc = ctx.enter_context(tc.tile_pool(name="ps_misc", bufs=1, space="PSUM"))

        # ---- weights on the gpsimd (SWDGE) queue, x inputs on sync ----
        iden_sb = consts.tile([128, 128], DT, tag="iden", name="iden_sb")
        nc.gpsimd.dma_start(out=iden_sb[:], in_=iden[:])
        wq_sb = [consts.tile([128, 256], DT, tag=f"wq{kc}", name=f"wq_sb{kc}") for kc in range(ECHUNK)]
        wk_sb = [consts.tile([128, 256], DT, tag=f"wk{kc}", name=f"wk_sb{kc}") for kc in range(ECHUNK)]
        wv_sb = [consts.tile([128, NH * D], DT, tag=f"wv{kc}", name=f"wv_sb{kc}") for kc in range(ECHUNK)]
        wo_sb = [consts.tile([128, E], DT, tag=f"wo{c}", name=f"wo_sb{c}") for c in range(2)]
        for kc in range(ECHUNK):
            nc.gpsimd.dma_start(out=wq_sb[kc][:], in_=wqtd[128 * kc:128 * (kc + 1), :])
            nc.gpsimd.dma_start(out=wk_sb[kc][:], in_=wktd[128 * kc:128 * (kc + 1), :])
        for kc in range(ECHUNK):
            nc.gpsimd.dma_start(out=wv_sb[kc][:], in_=wvt[128 * kc:128 * (kc + 1), :])
        for c in range(2):
            nc.gpsimd.dma_start(out=wo_sb[c][:], in_=wot[128 * c:128 * (c + 1), :])

        xq_sb = [big.tile([128, N], DT, tag="big", name="xq") for _ in range(ECHUNK)]
        xk_sb = [big.tile([128, N], DT, tag="big", name="xk") for _ in range(ECHUNK)]
        xv_sb = [big.tile([128, N], DT, tag="big", name="xv") for _ in range(ECHUNK)]

        def load_half(dst_tiles, src_ap, h):
            sl = slice(1024 * h, 1024 * (h + 1))
            for kc in range(ECHUNK):
                nc.sync.dma_start(
                    out=dst_tiles[kc][:, sl],
                    in_=src_ap[128 * kc:128 * (kc + 1), sl],
                )

        load_half(xq_sb, xqT, 0)
        load_half(xk_sb, xkT, 0)
        load_half(xq_sb, xqT, 1)
        load_half(xk_sb, xkT, 1)
        # xv rides the gpsimd queue after the weights: att@v(0) needs it
        # only ~10us after the first exp
        for kc in range(ECHUNK):
            nc.gpsimd.dma_start(out=xv_sb[kc][:], in_=xvT[128 * kc:128 * (kc + 1), :])

        # ---- q/k projections, ns-chunked; dup-swapped copies per chunk so
        # each head's 64 dims sit in BOTH partition halves: consecutive
        # energy matmuls then alternate PE row-groups, hiding LDWEIGHTS ----
        qnd = [qk.tile([128, N], DT, tag=f"qnd{mc}", name="qnd") for mc in range(2)]
        knd = [qk.tile([128, N], DT, tag=f"knd{mc}", name="knd") for mc in range(2)]
        qdp = [qk.tile([128, N], DT, tag=f"qdp{mc}", name="qdp") for mc in range(2)]
        kdp = [qk.tile([128, N], DT, tag=f"kdp{mc}", name="kdp") for mc in range(2)]

        proj_pools = [ps_misc, ps_energy]

        def emit_proj_chunk(w_sb, x_sb, nd, dp, mc, ns, pool):
            ps = pool.tile([128, 512], F32, tag="big" if pool is ps_energy else "misc", name="psp")
            for kc in range(ECHUNK):
                nc.tensor.matmul(
                    ps[:],
                    w_sb[kc][:, 128 * mc:128 * (mc + 1)],
                    x_sb[kc][:, 512 * ns:512 * (ns + 1)],
                    start=(kc == 0),
                    stop=(kc == ECHUNK - 1),
                )
            sl = slice(512 * ns, 512 * (ns + 1))
            nc.vector.tensor_copy(nd[mc][:, sl], ps[:])
            nc.gpsimd.dma_start(out=dp[mc][0:64, sl], in_=nd[mc][64:128, sl])
            nc.gpsimd.dma_start(out=dp[mc][64:128, sl], in_=nd[mc][0:64, sl])

        def half_ap(nd, dp, i, half):
            """[64, N] view of head i's projected data at partition `half`."""
            mc, r = divmod(i, 2)
            if half == 0:
                t = nd[mc] if r == 0 else dp[mc]
                return t[0:64, :]
            t = dp[mc] if r == 0 else nd[mc]
            return t[64:128, :]

        # mc0 q cols 0:1024 + k ns0 unblock the first energy->exp
        for ns in range(2):
            emit_proj_chunk(wq_sb, xq_sb, qnd, qdp, 0, ns, proj_pools[ns % 2])
        for ns in range(2):
            emit_proj_chunk(wk_sb, xk_sb, knd, kdp, 0, ns, proj_pools[ns % 2])
        for ns in range(2, 4):
            emit_proj_chunk(wq_sb, xq_sb, qnd, qdp, 0, ns, proj_pools[ns % 2])
        for ns in range(2, 4):
            emit_proj_chunk(wk_sb, xk_sb, knd, kdp, 0, ns, proj_pools[ns % 2])

        # ---- v projection (augmented ones column per head) ----
        vsb = [None] * NCHUNK

        def emit_vproj(mk):
            ps = ps_misc.tile([128, 512], F32, tag="misc", name="psv")
            for kc in range(ECHUNK):
                nc.tensor.matmul(
                    ps[:, 0:NH * D],
                    xv_sb[kc][:, 128 * mk:128 * (mk + 1)],
                    wv_sb[kc][:],
                    start=(kc == 0),
                    stop=(kc == ECHUNK - 1),
                )
            t = vp.tile([128, NH * 65], DT, tag=f"v{mk}", name=f"v_sb{mk}")
            vsrc = ps[:, 0:NH * D].rearrange("p (h d) -> p h d", h=NH)
            vdst = t[:].rearrange("p (h d) -> p h d", h=NH)[:, :, 0:D]
            nc.vector.tensor_copy(vdst, vsrc)
            ones_cols = t[:].rearrange("p (h d) -> p h d", h=NH)[:, :, D:D + 1]
            nc.vector.memset(ones_cols, 1.0)
            vsb[mk] = t

        # ---- attention state ----
        att = [[None] * NCHUNK for _ in range(NH)]   # attT fp16 [128, 2048] per (head, kc)
        pav = [None] * NH                            # (pavA, pavB, pavC) per head
        oall = [oallp.tile([128, NH * D], DT, tag=f"oall{m}", name=f"oall{m}") for m in range(NCHUNK)]
        ot = [otp.tile([128, N], DT, tag=f"ot{c}", name=f"ot{c}") for c in range(2)]

        def pav_slice(s, m):
            a, b, c = pav[s]
            if m < 7:
                return a, 65 * m
            if m < 14:
                return b, 65 * (m - 7)
            return c, 65 * (m - 14)

        def emit_energy(s, kc):
            t = big.tile([128, N], DT, tag="big", name="att")
            for half, ns in ((0, 0), (64, 1)):
                ps = ps_energy.tile([128, N // 2], F32, tag="big", name="ps")
                kh = half_ap(knd, kdp, s, half)
                qh = half_ap(qnd, qdp, s, half)
                for j in range(2):
                    nc.tensor.matmul(
                        ps[:, 512 * j:512 * (j + 1)],
                        kh[:, 128 * kc:128 * (kc + 1)],
                        qh[:, 1024 * ns + 512 * j:1024 * ns + 512 * (j + 1)],
                        start=True,
                        stop=True,
                    )
                nc.scalar.activation(
                    t[:, 1024 * ns:1024 * (ns + 1)], ps[:], AF.Exp, scale=SCALE
                )
            att[s][kc] = t

        def emit_attv_step(s, kc, m):
            pt, c = pav_slice(s, m)
            # one start per bank (zeros the whole 2KB zero region), one
            # stop per bank; middle writes accumulate
            first_in_bank = m in (0, 7, 14)
            last_in_bank = m in (6, 13, 15)
            nc.tensor.matmul(
                pt[:, c:c + 65],
                att[s][kc][:, 128 * m:128 * (m + 1)],
                vsb[kc][:, 65 * s:65 * s + 65],
                start=(kc == 0 and first_in_bank),
                stop=(kc == NCHUNK - 1 and last_in_bank),
                skip_group_check=True,
            )

        def emit_attv(s, kc):
            if kc == 0:
                pav[s] = (
                    ps_pavA.tile([128, 7 * 65], F32, tag="pav", name="pavA"),
                    ps_pavB.tile([128, 7 * 65], F32, tag="pav", name="pavB"),
                    ps_pavC.tile([128, 2 * 65], F32, tag="pav", name="pavC"),
                )
            for m in range(NCHUNK):
                emit_attv_step(s, kc, m)

        def emit_normalize_m(s, m):
            pt, c = pav_slice(s, m)
            rec = smallp.tile([128, 1], F32, tag="rec", name="rec")
            nc.vector.reciprocal(rec[:], pt[:, c + 64:c + 65])
            nc.vector.tensor_scalar_mul(
                oall[m][:, D * s:D * (s + 1)], pt[:, c:c + 64], rec[:]
            )

        def emit_normalize(s):
            for m in range(NCHUNK):
                emit_normalize_m(s, m)

        def emit_transpose(c, m, pool):
            pt = pool.tile([128, 128], DT, tag="misc" if pool is ps_misc else "big", name="pt")
            nc.tensor.transpose(pt[:], oall[m][:, 128 * c:128 * (c + 1)], iden_sb[:])
            nc.vector.tensor_copy(ot[c][:, 128 * m:128 * (m + 1)], pt[:])

        def emit_outproj(m):
            pf = ps_energy.tile([128, E], F32, tag="big", name="pff")
            for c in range(2):
                nc.tensor.matmul(
                    pf[:],
                    ot[c][:, 128 * m:128 * (m + 1)],
                    wo_sb[c][:],
                    start=(c == 0),
                    stop=(c == 1),
                )
            st = ostage.tile([128, E], DT, tag="st", name="st")
            nc.vector.tensor_copy(st[:], pf[:])
            nc.sync.dma_start(out=out[128 * m:128 * (m + 1), :], in_=st[:])

        # ---- unified (head, kc) stream with lagged att@v ----
        pending = []
        for g in range(NH * NCHUNK + LAG):
            s, kc = divmod(g, NCHUNK)
            if s < NH:
                emit_energy(s, kc)
            if s == 0:
                emit_vproj(kc)
            elif pending:
                pending.pop(0)()
            if g == NCHUNK:
                # second head-pair projections, spread across stage 1
                for ns in range(4):
                    pending.append(
                        lambda ns=ns: emit_proj_chunk(wq_sb, xq_sb, qnd, qdp, 1, ns, ps_misc))
                for ns in range(4):
                    pending.append(
                        lambda ns=ns: emit_proj_chunk(wk_sb, xk_sb, knd, kdp, 1, ns, ps_misc))
            ga = g - LAG
            if ga >= 0:
                sa, kca = divmod(ga, NCHUNK)
                emit_attv(sa, kca)
                if kca == NCHUNK - 1:
                    emit_normalize(sa)
                    if sa == 1:
                        # heads 0,1 done: transpose pair 01, spread out
                        for m in range(NCHUNK):
                            pending.append(lambda m=m: emit_transpose(0, m, ps_misc))

        # ---- tail: pair 23 transpose + out-proj + fp16 output DMA ----
        for fn in pending:
            fn()
        for m in range(NCHUNK):
            emit_transpose(1, m, ps_misc)
            emit_outproj(m)

    nc.compile()
    _built = nc
    return nc


def _host_prep(query, key, value, Wq, Wk, Wv, Wo, c):
    b, g = c // 2, c % 2
    DT = np.float16
    wqtd = np.empty((E, 256), np.float32)
    wktd = np.empty((E, 256), np.float32)
    wvt = np.empty((E, NH * D), np.float32)
    wot = np.empty((NH * D, E), np.float32)
    for i in range(NH):
        h = NH * g + i
        wqtd[:, D * i:D * (i + 1)] = Wq[D * h:D * (h + 1), :].T
        wktd[:, D * i:D * (i + 1)] = Wk[D * h:D * (h + 1), :].T
        wvt[:, D * i:D * (i + 1)] = Wv[D * h:D * (h + 1), :].T
        wot[D * i:D * (i + 1), :] = Wo[:, D * h:D * (h + 1)].T
    return {
        "xqT": np.ascontiguousarray(query[b].T).astype(DT),
        "xkT": np.ascontiguousarray(key[b].T).astype(DT),
        "xvT": np.ascontiguousarray(value[b].T).astype(DT),
        "wqtd": wqtd.astype(DT),
        "wktd": wktd.astype(DT),
        "wvt": wvt.astype(DT),
        "wot": wot.astype(DT),
        "iden": np.eye(128, dtype=DT),
    }


# test.py can flip these to profile
TRACE = False
TRACE_KWARGS = {}
LAST_RESULTS = None


def kernel(query, key, value, Wq, Wk, Wv, Wo):
    global LAST_RESULTS
    from concourse.bass_utils import run_bass_kernel_spmd

    args = [np.asarray(x, dtype=np.float32) for x in (query, key, value, Wq, Wk, Wv, Wo)]
    nc = _build()
    in_maps = [_host_prep(*args, c) for c in range(N_CORES)]
    res = run_bass_kernel_spmd(
        nc, in_maps, core_ids=list(range(N_CORES)), trace=TRACE, **TRACE_KWARGS
    )
    LAST_RESULTS = res
    outp = np.zeros((B, N, E), np.float32)
    for c in range(N_CORES):
        outp[c // 2] += res.results[c]["out"].astype(np.float32)
    return outp


# revision 18
# speedup vs baseline: 1.1007x; 1.1007x over previous
"""Multi-head attention (B=4, N=2048, E=512, H=8) on 8 Trainium2 cores.

Sharding: core c -> (batch b = c//2, head-group g = c%2 of 4 heads).
Each core computes q/k/v projections for its 4 heads, full attention,
and a partial output projection (its heads' slice of Wo contraction);
the host sums the two partials per batch (fp16 partials).

v3 (spread att@v; ACT-saturated pipeline):
  - unified stream over (head, kchunk): PE emits energy(s,kc) -> ACT exp
    -> 4-kc-lagged att@v accumulation steps for the same head, so att@v
    never forms a serial tail (the old tail was ~75us at half clock)
  - att@v accumulates into packed PSUM tiles ([128,455]x2 + [128,130],
    16 q-chunks x 65 cols each, bank-straddle-free) across all 16 kc
  - per-q-chunk denominator comes from the ones-column of v_aug as
    before; normalization (DVE) right after each head's last att@v step
  - head-pair transposes for the output projection run early (pair 01
    during head 2's stage); only pair 23 + out-proj + fp16 output DMA
    remain in the ~10us tail
"""

import sys

if "/opt/trn_rl_repo" not in sys.path:
    sys.path.insert(0, "/opt/trn_rl_repo")

import numpy as np

B, N, E, H, D = 4, 2048, 512, 8, 64
NH = 4                      # heads per core
NCHUNK = N // 128           # 16
ECHUNK = E // 128           # 4
SCALE = float(1.0 / np.sqrt(E))
N_CORES = 8
LAG = 4                     # att@v trails energy by LAG kchunks

ATT_POOL_BUFS = 34          # shared [128,2048] fp16 slots: 12 xT tiles + 2-head attT window

_built = None


def _build():
    global _built
    if _built is not None:
        return _built

    from contextlib import ExitStack

    import concourse.bass as bass  # noqa: F401
    import concourse.mybir as mybir
    import concourse.tile as tile
    from concourse import bacc

    DT = mybir.dt.float16
    F32 = mybir.dt.float32
    AF = mybir.ActivationFunctionType

    nc = bacc.Bacc(
        "TRN2",
        target_bir_lowering=False,
        debug=False,
        num_devices=N_CORES,
    )

    xqT = nc.dram_tensor("xqT", [E, N], DT, kind="ExternalInput").ap()
    xkT = nc.dram_tensor("xkT", [E, N], DT, kind="ExternalInput").ap()
    xvT = nc.dram_tensor("xvT", [E, N], DT, kind="ExternalInput").ap()
    wqtd = nc.dram_tensor("wqtd", [E, 256], DT, kind="ExternalInput").ap()
    wktd = nc.dram_tensor("wktd", [E, 256], DT, kind="ExternalInput").ap()
    wvt = nc.dram_tensor("wvt", [E, NH * D], DT, kind="ExternalInput").ap()
    wot = nc.dram_tensor("wot", [NH * D, E], DT, kind="ExternalInput").ap()
    iden = nc.dram_tensor("iden", [128, 128], DT, kind="ExternalInput").ap()
    out = nc.dram_tensor("out", [N, E], DT, kind="ExternalOutput").ap()

    with tile.TileContext(nc) as tc, ExitStack() as ctx:
        consts = ctx.enter_context(tc.tile_pool(name="consts", bufs=1))
        big = ctx.enter_context(tc.tile_pool(name="big", bufs=ATT_POOL_BUFS))
        qk = ctx.enter_context(tc.tile_pool(name="qk", bufs=1))
        vp = ctx.enter_context(tc.tile_pool(name="vp", bufs=1))
        oallp = ctx.enter_context(tc.tile_pool(name="oall", bufs=1))
        otp = ctx.enter_context(tc.tile_pool(name="ot", bufs=1))
        ostage = ctx.enter_context(tc.tile_pool(name="ostage", bufs=3))
        smallp = ctx.enter_context(tc.tile_pool(name="small", bufs=4))

        # PSUM (8 banks): energy f32 [128,1024] x2 bufs (4 banks),
        # att@v accumulators 3 single-buf pools (1 bank each), misc 1 bank.
        # NOTE: matmul start=True zeros the whole 2KB bank (zero region), so
        # each pav bank gets exactly one start (first region) and one stop
        # (last region) per accumulation pass.
        ps_energy = ctx.enter_context(tc.tile_pool(name="ps_energy", bufs=2, space="PSUM"))
        ps_pavA = ctx.enter_context(tc.tile_pool(name="ps_pavA", bufs=1, space="PSUM"))
        ps_pavB = ctx.enter_context(tc.tile_pool(name="ps_pavB", bufs=1, space="PSUM"))
        ps_pavC = ctx.enter_context(tc.tile_pool(name="ps_pavC", bufs=1, space="PSUM"))
        ps_misc = ctx.enter_context(tc.tile_pool(name="ps_misc", bufs=1, space="PSUM"))

        # ---- weights on the gpsimd (SWDGE) queue, x inputs on sync ----
        iden_sb = consts.tile([128, 128], DT, tag="iden", name="iden_sb")
        nc.gpsimd.dma_start(out=iden_sb[:], in_=iden[:])
        wq_sb = [consts.tile([128, 256], DT, tag=f"wq{kc}", name=f"wq_sb{kc}") for kc in range(ECHUNK)]
        wk_sb = [consts.tile([128, 256], DT, tag=f"wk{kc}", name=f"wk_sb{kc}") for kc in range(ECHUNK)]
        wv_sb = [consts.tile([128, NH * D], DT, tag=f"wv{kc}", name=f"wv_sb{kc}") for kc in range(ECHUNK)]
        wo_sb = [consts.tile([128, E], DT, tag=f"wo{c}", name=f"wo_sb{c}") for c in range(2)]
        for kc in range(ECHUNK):
            nc.gpsimd.dma_start(out=wq_sb[kc][:], in_=wqtd[128 * kc:128 * (kc + 1), :])
            nc.gpsimd.dma_start(out=wk_sb[kc][:], in_=wktd[128 * kc:128 * (kc + 1), :])
        for kc in range(ECHUNK):
            nc.gpsimd.dma_start(out=wv_sb[kc][:], in_=wvt[128 * kc:128 * (kc + 1), :])
        for c in range(2):
            nc.gpsimd.dma_start(out=wo_sb[c][:], in_=wot[128 * c:128 * (c + 1), :])

        xq_sb = [big.tile([128, N], DT, tag="big", name="xq") for _ in range(ECHUNK)]
        xk_sb = [big.tile([128, N], DT, tag="big", name="xk") for _ in range(ECHUNK)]
        xv_sb = [big.tile([128, N], DT, tag="big", name="xv") for _ in range(ECHUNK)]

        def load_half(dst_tiles, src_ap, h):
            sl = slice(1024 * h, 1024 * (h + 1))
            for kc in range(ECHUNK):
                nc.sync.dma_start(
                    out=dst_tiles[kc][:, sl],
                    in_=src_ap[128 * kc:128 * (kc + 1), sl],
                )

        load_half(xq_sb, xqT, 0)
        load_half(xk_sb, xkT, 0)
        load_half(xv_sb, xvT, 0)
        load_half(xq_sb, xqT, 1)
        load_half(xk_sb, xkT, 1)
        load_half(xv_sb, xvT, 1)

        # ---- q/k projections, ns-chunked; dup-swapped copies per chunk so
        # each head's 64 dims sit in BOTH partition halves: consecutive
        # energy matmuls then alternate PE row-groups, hiding LDWEIGHTS ----
        qnd = [qk.tile([128, N], DT, tag=f"qnd{mc}", name="qnd") for mc in range(2)]
        knd = [qk.tile([128, N], DT, tag=f"knd{mc}", name="knd") for mc in range(2)]
        qdp = [qk.tile([128, N], DT, tag=f"qdp{mc}", name="qdp") for mc in range(2)]
        kdp = [qk.tile([128, N], DT, tag=f"kdp{mc}", name="kdp") for mc in range(2)]

        proj_pools = [ps_misc, ps_energy]

        def emit_proj_chunk(w_sb, x_sb, nd, dp, mc, ns, pool):
            ps = pool.tile([128, 512], F32, tag="big" if pool is ps_energy else "misc", name="psp")
            for kc in range(ECHUNK):
                nc.tensor.matmul(
                    ps[:],
                    w_sb[kc][:, 128 * mc:128 * (mc + 1)],
                    x_sb[kc][:, 512 * ns:512 * (ns + 1)],
                    start=(kc == 0),
                    stop=(kc == ECHUNK - 1),
                )
            sl = slice(512 * ns, 512 * (ns + 1))
            nc.vector.tensor_copy(nd[mc][:, sl], ps[:])
            nc.gpsimd.dma_start(out=dp[mc][0:64, sl], in_=nd[mc][64:128, sl])
            nc.gpsimd.dma_start(out=dp[mc][64:128, sl], in_=nd[mc][0:64, sl])

        def half_ap(nd, dp, i, half):
            """[64, N] view of head i's projected data at partition `half`."""
            mc, r = divmod(i, 2)
            if half == 0:
                t = nd[mc] if r == 0 else dp[mc]
                return t[0:64, :]
            t = dp[mc] if r == 0 else nd[mc]
            return t[64:128, :]

        # mc0 q cols 0:1024 + k ns0 unblock the first energy->exp
        for ns in range(2):
            emit_proj_chunk(wq_sb, xq_sb, qnd, qdp, 0, ns, proj_pools[ns % 2])
        for ns in range(2):
            emit_proj_chunk(wk_sb, xk_sb, knd, kdp, 0, ns, proj_pools[ns % 2])
        for ns in range(2, 4):
            emit_proj_chunk(wq_sb, xq_sb, qnd, qdp, 0, ns, proj_pools[ns % 2])
        for ns in range(2, 4):
            emit_proj_chunk(wk_sb, xk_sb, knd, kdp, 0, ns, proj_pools[ns % 2])

        # ---- v projection (augmented ones column per head) ----
        vsb = [None] * NCHUNK

        def emit_vproj(mk):
            ps = ps_misc.tile([128, 512], F32, tag="misc", name="psv")
            for kc in range(ECHUNK):
                nc.tensor.matmul(
                    ps[:, 0:NH * D],
                    xv_sb[kc][:, 128 * mk:128 * (mk + 1)],
                    wv_sb[kc][:],
                    start=(kc == 0),
                    stop=(kc == ECHUNK - 1),
                )
            t = vp.tile([128, NH * 65], DT, tag=f"v{mk}", name=f"v_sb{mk}")
            vsrc = ps[:, 0:NH * D].rearrange("p (h d) -> p h d", h=NH)
            vdst = t[:].rearrange("p (h d) -> p h d", h=NH)[:, :, 0:D]
            nc.vector.tensor_copy(vdst, vsrc)
            ones_cols = t[:].rearrange("p (h d) -> p h d", h=NH)[:, :, D:D + 1]
            nc.vector.memset(ones_cols, 1.0)
            vsb[mk] = t

        # ---- attention state ----
        att = [[None] * NCHUNK for _ in range(NH)]   # attT fp16 [128, 2048] per (head, kc)
        pav = [None] * NH                            # (pavA, pavB, pavC) per head
        oall = [oallp.tile([128, NH * D], DT, tag=f"oall{m}", name=f"oall{m}") for m in range(NCHUNK)]
        ot = [otp.tile([128, N], DT, tag=f"ot{c}", name=f"ot{c}") for c in range(2)]

        def pav_slice(s, m):
            a, b, c = pav[s]
            if m < 7:
                return a, 65 * m
            if m < 14:
                return b, 65 * (m - 7)
            return c, 65 * (m - 14)

        def emit_energy(s, kc):
            t = big.tile([128, N], DT, tag="big", name="att")
            for half, ns in ((0, 0), (64, 1)):
                ps = ps_energy.tile([128, N // 2], F32, tag="big", name="ps")
                kh = half_ap(knd, kdp, s, half)
                qh = half_ap(qnd, qdp, s, half)
                for j in range(2):
                    nc.tensor.matmul(
                        ps[:, 512 * j:512 * (j + 1)],
                        kh[:, 128 * kc:128 * (kc + 1)],
                        qh[:, 1024 * ns + 512 * j:1024 * ns + 512 * (j + 1)],
                        start=True,
                        stop=True,
                    )
                nc.scalar.activation(
                    t[:, 1024 * ns:1024 * (ns + 1)], ps[:], AF.Exp, scale=SCALE
                )
            att[s][kc] = t

        def emit_attv_step(s, kc, m):
            pt, c = pav_slice(s, m)
            # one start per bank (zeros the whole 2KB zero region), one
            # stop per bank; middle writes accumulate
            first_in_bank = m in (0, 7, 14)
            last_in_bank = m in (6, 13, 15)
            nc.tensor.matmul(
                pt[:, c:c + 65],
                att[s][kc][:, 128 * m:128 * (m + 1)],
                vsb[kc][:, 65 * s:65 * s + 65],
                start=(kc == 0 and first_in_bank),
                stop=(kc == NCHUNK - 1 and last_in_bank),
                skip_group_check=True,
            )

        def emit_attv(s, kc):
            if kc == 0:
                pav[s] = (
                    ps_pavA.tile([128, 7 * 65], F32, tag="pav", name="pavA"),
                    ps_pavB.tile([128, 7 * 65], F32, tag="pav", name="pavB"),
                    ps_pavC.tile([128, 2 * 65], F32, tag="pav", name="pavC"),
                )
            for m in range(NCHUNK):
                emit_attv_step(s, kc, m)

        def emit_normalize_m(s, m):
            pt, c = pav_slice(s, m)
            rec = smallp.tile([128, 1], F32, tag="rec", name="rec")
            nc.vector.reciprocal(rec[:], pt[:, c + 64:c + 65])
            nc.vector.tensor_scalar_mul(
                oall[m][:, D * s:D * (s + 1)], pt[:, c:c + 64], rec[:]
            )

        def emit_normalize(s):
            for m in range(NCHUNK):
                emit_normalize_m(s, m)

        def emit_transpose(c, m, pool):
            pt = pool.tile([128, 128], DT, tag="misc" if pool is ps_misc else "big", name="pt")
            nc.tensor.transpose(pt[:], oall[m][:, 128 * c:128 * (c + 1)], iden_sb[:])
            nc.vector.tensor_copy(ot[c][:, 128 * m:128 * (m + 1)], pt[:])

        def emit_outproj(m):
            pf = ps_energy.tile([128, E], F32, tag="big", name="pff")
            for c in range(2):
                nc.tensor.matmul(
                    pf[:],
                    ot[c][:, 128 * m:128 * (m + 1)],
                    wo_sb[c][:],
                    start=(c == 0),
                    stop=(c == 1),
                )
            st = ostage.tile([128, E], DT, tag="st", name="st")
            nc.vector.tensor_copy(st[:], pf[:])
            nc.sync.dma_start(out=out[128 * m:128 * (m + 1), :], in_=st[:])

        # ---- unified (head, kc) stream with lagged att@v ----
        pending = []
        for g in range(NH * NCHUNK + LAG):
            s, kc = divmod(g, NCHUNK)
            if s < NH:
                emit_energy(s, kc)
            if s == 0:
                emit_vproj(kc)
            elif pending:
                pending.pop(0)()
            if g == NCHUNK:
                # second head-pair projections, spread across stage 1
                for ns in range(4):
                    pending.append(
                        lambda ns=ns: emit_proj_chunk(wq_sb, xq_sb, qnd, qdp, 1, ns, ps_misc))
                for ns in range(4):
                    pending.append(
                        lambda ns=ns: emit_proj_chunk(wk_sb, xk_sb, knd, kdp, 1, ns, ps_misc))
            ga = g - LAG
            if ga >= 0:
                sa, kca = divmod(ga, NCHUNK)
                emit_attv(sa, kca)
                if kca == NCHUNK - 1:
                    emit_normalize(sa)
                    if sa == 1:
                        # heads 0,1 done: transpose pair 01, spread out
                        for m in range(NCHUNK):
                            pending.append(lambda m=m: emit_transpose(0, m, ps_misc))

        # ---- tail: pair 23 transpose + out-proj + fp16 output DMA ----
        for fn in pending:
            fn()
        for m in range(NCHUNK):
            emit_transpose(1, m, ps_misc)
            emit_outproj(m)

    nc.compile()
    _built = nc
    return nc


def _host_prep(query, key, value, Wq, Wk, Wv, Wo, c):
    b, g = c // 2, c % 2
    DT = np.float16
    wqtd = np.empty((E, 256), np.float32)
    wktd = np.empty((E, 256), np.float32)
    wvt = np.empty((E, NH * D), np.float32)
    wot = np.empty((NH * D, E), np.float32)
    for i in range(NH):
        h = NH * g + i
        wqtd[:, D * i:D * (i + 1)] = Wq[D * h:D * (h + 1), :].T
        wktd[:, D * i:D * (i + 1)] = Wk[D * h:D * (h + 1), :].T
        wvt[:, D * i:D * (i + 1)] = Wv[D * h:D * (h + 1), :].T
        wot[D * i:D * (i + 1), :] = Wo[:, D * h:D * (h + 1)].T
    return {
        "xqT": np.ascontiguousarray(query[b].T).astype(DT),
        "xkT": np.ascontiguousarray(key[b].T).astype(DT),
        "xvT": np.ascontiguousarray(value[b].T).astype(DT),
        "wqtd": wqtd.astype(DT),
        "wktd": wktd.astype(DT),
        "wvt": wvt.astype(DT),
        "wot": wot.astype(DT),
        "iden": np.eye(128, dtype=DT),
    }


# test.py can flip these to profile
TRACE = False
TRACE_KWARGS = {}
LAST_RESULTS = None


def kernel(query, key, value, Wq, Wk, Wv, Wo):
    global LAST_RESULTS
    from concourse.bass_utils import run_bass_kernel_spmd

    args = [np.asarray(x, dtype=np.float32) for x in (query, key, value, Wq, Wk, Wv, Wo)]
    nc = _build()
    in_maps = [_host_prep(*args, c) for c in range(N_CORES)]
    res = run_bass_kernel_spmd(
        nc, in_maps, core_ids=list(range(N_CORES)), trace=TRACE, **TRACE_KWARGS
    )
    LAST_RESULTS = res
    outp = np.zeros((B, N, E), np.float32)
    for c in range(N_CORES):
        outp[c // 2] += res.results[c]["out"].astype(np.float32)
    return outp


# revision 20
# speedup vs baseline: 1.1161x; 1.0139x over previous
"""Multi-head attention (B=4, N=2048, E=512, H=8) on 8 Trainium2 cores.

Sharding: core c -> (batch b = c//2, head-group g = c%2 of 4 heads).
Each core computes q/k/v projections for its 4 heads, full attention,
and a partial output projection (its heads' slice of Wo contraction);
the host sums the two partials per batch (fp16 partials).

Design (spread att@v; exp-saturated pipeline):
  - unified stream over (head, kchunk): PE emits energy(s,kc) -> ACT exp
    -> 4-kc-lagged att@v accumulation steps for the same head, so att@v
    never forms a big serial tail (the old per-head att@v block was
    ~50-75us at HAM half clock)
  - att@v accumulates into packed PSUM tiles ([128,455]x2 + [128,130]:
    16 q-chunks x 65 cols, regions 256B-aligned within banks) across all
    16 kchunks; exactly ONE matmul start per 2KB PSUM bank (start zeros
    the whole zero region) and one stop, middle steps accumulate
  - per-q-chunk softmax denominator comes from the ones-column of v_aug;
    normalization (DVE reciprocal + scalar-mul) right after each head's
    last att@v step
  - q/k dup-swapped copies keep each head's 64 dims in BOTH partition
    halves so consecutive energy matmuls alternate PE row-groups (hides
    LDWEIGHTS; measured 216ns cadence for 512-col matmuls at 2.4GHz)
  - column-half input DMAs + ns-chunked projections start the first exp
    ~30us in; mc1 projections and pair-01 transposes drain one-per-step
    from a pending queue so bursts never stall the in-order PE stream
  - tail: pair-23 transposes + out-proj + fp16 partial output DMA
"""

import sys

if "/opt/trn_rl_repo" not in sys.path:
    sys.path.insert(0, "/opt/trn_rl_repo")

import numpy as np

B, N, E, H, D = 4, 2048, 512, 8, 64
NH = 4                      # heads per core
NCHUNK = N // 128           # 16
ECHUNK = E // 128           # 4
SCALE = float(1.0 / np.sqrt(E))
N_CORES = 8
LAG = 4                     # att@v trails energy by LAG kchunks

ATT_POOL_BUFS = 34          # shared [128,2048] fp16 slots: 12 xT tiles + 2-head attT window

_built = None


def _build():
    global _built
    if _built is not None:
        return _built

    from contextlib import ExitStack

    import concourse.bass as bass  # noqa: F401
    import concourse.mybir as mybir
    import concourse.tile as tile
    from concourse import bacc

    DT = mybir.dt.float16
    F32 = mybir.dt.float32
    AF = mybir.ActivationFunctionType

    nc = bacc.Bacc(
        "TRN2",
        target_bir_lowering=False,
        debug=False,
        num_devices=N_CORES,
    )

    xqT = nc.dram_tensor("xqT", [E, N], DT, kind="ExternalInput").ap()
    xkT = nc.dram_tensor("xkT", [E, N], DT, kind="ExternalInput").ap()
    xvT = nc.dram_tensor("xvT", [E, N], DT, kind="ExternalInput").ap()
    wqtd = nc.dram_tensor("wqtd", [E, 256], DT, kind="ExternalInput").ap()
    wktd = nc.dram_tensor("wktd", [E, 256], DT, kind="ExternalInput").ap()
    wvt = nc.dram_tensor("wvt", [E, NH * D], DT, kind="ExternalInput").ap()
    wot = nc.dram_tensor("wot", [NH * D, E], DT, kind="ExternalInput").ap()
    iden = nc.dram_tensor("iden", [128, 128], DT, kind="ExternalInput").ap()
    out = nc.dram_tensor("out", [N, E], DT, kind="ExternalOutput").ap()

    with tile.TileContext(nc) as tc, ExitStack() as ctx:
        consts = ctx.enter_context(tc.tile_pool(name="consts", bufs=1))
        big = ctx.enter_context(tc.tile_pool(name="big", bufs=ATT_POOL_BUFS))
        qk = ctx.enter_context(tc.tile_pool(name="qk", bufs=1))
        vp = ctx.enter_context(tc.tile_pool(name="vp", bufs=1))
        oallp = ctx.enter_context(tc.tile_pool(name="oall", bufs=1))
        otp = ctx.enter_context(tc.tile_pool(name="ot", bufs=1))
        ostage = ctx.enter_context(tc.tile_pool(name="ostage", bufs=3))
        smallp = ctx.enter_context(tc.tile_pool(name="small", bufs=4))

        # PSUM (8 banks): energy f32 [128,1024] x2 bufs (4 banks),
        # att@v accumulators 3 single-buf pools (1 bank each), misc 1 bank.
        # NOTE: matmul start=True zeros the whole 2KB bank (zero region), so
        # each pav bank gets exactly one start (first region) and one stop
        # (last region) per accumulation pass.
        ps_energy = ctx.enter_context(tc.tile_pool(name="ps_energy", bufs=2, space="PSUM"))
        ps_pavA = ctx.enter_context(tc.tile_pool(name="ps_pavA", bufs=1, space="PSUM"))
        ps_pavB = ctx.enter_context(tc.tile_pool(name="ps_pavB", bufs=1, space="PSUM"))
        ps_pavC = ctx.enter_context(tc.tile_pool(name="ps_pavC", bufs=1, space="PSUM"))
        ps_misc = ctx.enter_context(tc.tile_pool(name="ps_misc", bufs=1, space="PSUM"))

        # ---- weights on the gpsimd (SWDGE) queue, x inputs on sync ----
        iden_sb = consts.tile([128, 128], DT, tag="iden", name="iden_sb")
        nc.gpsimd.dma_start(out=iden_sb[:], in_=iden[:])
        wq_sb = [consts.tile([128, 256], DT, tag=f"wq{kc}", name=f"wq_sb{kc}") for kc in range(ECHUNK)]
        wk_sb = [consts.tile([128, 256], DT, tag=f"wk{kc}", name=f"wk_sb{kc}") for kc in range(ECHUNK)]
        wv_sb = [consts.tile([128, NH * D], DT, tag=f"wv{kc}", name=f"wv_sb{kc}") for kc in range(ECHUNK)]
        wo_sb = [consts.tile([128, E], DT, tag=f"wo{c}", name=f"wo_sb{c}") for c in range(2)]
        for kc in range(ECHUNK):
            nc.gpsimd.dma_start(out=wq_sb[kc][:], in_=wqtd[128 * kc:128 * (kc + 1), :])
            nc.gpsimd.dma_start(out=wk_sb[kc][:], in_=wktd[128 * kc:128 * (kc + 1), :])
        for kc in range(ECHUNK):
            nc.gpsimd.dma_start(out=wv_sb[kc][:], in_=wvt[128 * kc:128 * (kc + 1), :])
        for c in range(2):
            nc.gpsimd.dma_start(out=wo_sb[c][:], in_=wot[128 * c:128 * (c + 1), :])

        xq_sb = [big.tile([128, N], DT, tag="big", name="xq") for _ in range(ECHUNK)]
        xk_sb = [big.tile([128, N], DT, tag="big", name="xk") for _ in range(ECHUNK)]
        xv_sb = [big.tile([128, N], DT, tag="big", name="xv") for _ in range(ECHUNK)]

        def load_half(dst_tiles, src_ap, h):
            sl = slice(1024 * h, 1024 * (h + 1))
            for kc in range(ECHUNK):
                nc.sync.dma_start(
                    out=dst_tiles[kc][:, sl],
                    in_=src_ap[128 * kc:128 * (kc + 1), sl],
                )

        load_half(xq_sb, xqT, 0)
        load_half(xk_sb, xkT, 0)
        load_half(xv_sb, xvT, 0)
        load_half(xq_sb, xqT, 1)
        load_half(xk_sb, xkT, 1)
        load_half(xv_sb, xvT, 1)

        # ---- q/k projections, ns-chunked; dup-swapped copies per chunk so
        # each head's 64 dims sit in BOTH partition halves: consecutive
        # energy matmuls then alternate PE row-groups, hiding LDWEIGHTS ----
        qnd = [qk.tile([128, N], DT, tag=f"qnd{mc}", name="qnd") for mc in range(2)]
        knd = [qk.tile([128, N], DT, tag=f"knd{mc}", name="knd") for mc in range(2)]
        qdp = [qk.tile([128, N], DT, tag=f"qdp{mc}", name="qdp") for mc in range(2)]
        kdp = [qk.tile([128, N], DT, tag=f"kdp{mc}", name="kdp") for mc in range(2)]

        proj_pools = [ps_misc, ps_energy]

        def emit_proj_chunk(w_sb, x_sb, nd, dp, mc, ns, pool):
            ps = pool.tile([128, 512], F32, tag="big" if pool is ps_energy else "misc", name="psp")
            for kc in range(ECHUNK):
                nc.tensor.matmul(
                    ps[:],
                    w_sb[kc][:, 128 * mc:128 * (mc + 1)],
                    x_sb[kc][:, 512 * ns:512 * (ns + 1)],
                    start=(kc == 0),
                    stop=(kc == ECHUNK - 1),
                )
            sl = slice(512 * ns, 512 * (ns + 1))
            nc.vector.tensor_copy(nd[mc][:, sl], ps[:])
            nc.gpsimd.dma_start(out=dp[mc][0:64, sl], in_=nd[mc][64:128, sl])
            nc.gpsimd.dma_start(out=dp[mc][64:128, sl], in_=nd[mc][0:64, sl])

        def half_ap(nd, dp, i, half):
            """[64, N] view of head i's projected data at partition `half`."""
            mc, r = divmod(i, 2)
            if half == 0:
                t = nd[mc] if r == 0 else dp[mc]
                return t[0:64, :]
            t = dp[mc] if r == 0 else nd[mc]
            return t[64:128, :]

        # mc0 q cols 0:1024 + k ns0 unblock the first energy->exp
        for ns in range(2):
            emit_proj_chunk(wq_sb, xq_sb, qnd, qdp, 0, ns, proj_pools[ns % 2])
        for ns in range(2):
            emit_proj_chunk(wk_sb, xk_sb, knd, kdp, 0, ns, proj_pools[ns % 2])
        for ns in range(2, 4):
            emit_proj_chunk(wq_sb, xq_sb, qnd, qdp, 0, ns, proj_pools[ns % 2])
        for ns in range(2, 4):
            emit_proj_chunk(wk_sb, xk_sb, knd, kdp, 0, ns, proj_pools[ns % 2])

        # ---- v projection (augmented ones column per head) ----
        vsb = [None] * NCHUNK

        def emit_vproj(mk):
            ps = ps_misc.tile([128, 512], F32, tag="misc", name="psv")
            for kc in range(ECHUNK):
                nc.tensor.matmul(
                    ps[:, 0:NH * D],
                    xv_sb[kc][:, 128 * mk:128 * (mk + 1)],
                    wv_sb[kc][:],
                    start=(kc == 0),
                    stop=(kc == ECHUNK - 1),
                )
            t = vp.tile([128, NH * 65], DT, tag=f"v{mk}", name=f"v_sb{mk}")
            vsrc = ps[:, 0:NH * D].rearrange("p (h d) -> p h d", h=NH)
            vdst = t[:].rearrange("p (h d) -> p h d", h=NH)[:, :, 0:D]
            nc.vector.tensor_copy(vdst, vsrc)
            ones_cols = t[:].rearrange("p (h d) -> p h d", h=NH)[:, :, D:D + 1]
            nc.vector.memset(ones_cols, 1.0)
            vsb[mk] = t

        # ---- attention state ----
        att = [[None] * NCHUNK for _ in range(NH)]   # attT fp16 [128, 2048] per (head, kc)
        pav = [None] * NH                            # (pavA, pavB, pavC) per head
        oall = [oallp.tile([128, NH * D], DT, tag=f"oall{m}", name=f"oall{m}") for m in range(NCHUNK)]
        ot = [otp.tile([128, N], DT, tag=f"ot{c}", name=f"ot{c}") for c in range(2)]

        def pav_slice(s, m):
            a, b, c = pav[s]
            if m < 7:
                return a, 65 * m
            if m < 14:
                return b, 65 * (m - 7)
            return c, 65 * (m - 14)

        def emit_energy(s, kc):
            t = big.tile([128, N], DT, tag="big", name="att")
            for half, ns in ((0, 0), (64, 1)):
                ps = ps_energy.tile([128, N // 2], F32, tag="big", name="ps")
                kh = half_ap(knd, kdp, s, half)
                qh = half_ap(qnd, qdp, s, half)
                for j in range(2):
                    nc.tensor.matmul(
                        ps[:, 512 * j:512 * (j + 1)],
                        kh[:, 128 * kc:128 * (kc + 1)],
                        qh[:, 1024 * ns + 512 * j:1024 * ns + 512 * (j + 1)],
                        start=True,
                        stop=True,
                    )
                nc.scalar.activation(
                    t[:, 1024 * ns:1024 * (ns + 1)], ps[:], AF.Exp, scale=SCALE
                )
            att[s][kc] = t

        def emit_attv_step(s, kc, m):
            pt, c = pav_slice(s, m)
            # one start per bank (zeros the whole 2KB zero region), one
            # stop per bank; middle writes accumulate
            first_in_bank = m in (0, 7, 14)
            last_in_bank = m in (6, 13, 15)
            nc.tensor.matmul(
                pt[:, c:c + 65],
                att[s][kc][:, 128 * m:128 * (m + 1)],
                vsb[kc][:, 65 * s:65 * s + 65],
                start=(kc == 0 and first_in_bank),
                stop=(kc == NCHUNK - 1 and last_in_bank),
                skip_group_check=True,
            )

        def emit_attv(s, kc):
            if kc == 0:
                pav[s] = (
                    ps_pavA.tile([128, 7 * 65], F32, tag="pav", name="pavA"),
                    ps_pavB.tile([128, 7 * 65], F32, tag="pav", name="pavB"),
                    ps_pavC.tile([128, 2 * 65], F32, tag="pav", name="pavC"),
                )
            for m in range(NCHUNK):
                emit_attv_step(s, kc, m)

        def emit_normalize_m(s, m):
            pt, c = pav_slice(s, m)
            rec = smallp.tile([128, 1], F32, tag="rec", name="rec")
            nc.vector.reciprocal(rec[:], pt[:, c + 64:c + 65])
            nc.vector.tensor_scalar_mul(
                oall[m][:, D * s:D * (s + 1)], pt[:, c:c + 64], rec[:]
            )

        def emit_normalize(s):
            for m in range(NCHUNK):
                emit_normalize_m(s, m)

        def emit_transpose(c, m, pool):
            pt = pool.tile([128, 128], DT, tag="misc" if pool is ps_misc else "big", name="pt")
            nc.tensor.transpose(pt[:], oall[m][:, 128 * c:128 * (c + 1)], iden_sb[:])
            nc.vector.tensor_copy(ot[c][:, 128 * m:128 * (m + 1)], pt[:])

        def emit_outproj(m):
            pf = ps_energy.tile([128, E], F32, tag="big", name="pff")
            for c in range(2):
                nc.tensor.matmul(
                    pf[:],
                    ot[c][:, 128 * m:128 * (m + 1)],
                    wo_sb[c][:],
                    start=(c == 0),
                    stop=(c == 1),
                )
            st = ostage.tile([128, E], DT, tag="st", name="st")
            nc.vector.tensor_copy(st[:], pf[:])
            nc.sync.dma_start(out=out[128 * m:128 * (m + 1), :], in_=st[:])

        # ---- unified (head, kc) stream with lagged att@v ----
        # att@v group a (0..63) runs at g = a + lag(a): lag 4 in steady
        # state, ramping to 1 across the last head so only one kc-group of
        # att@v trails the final exp
        lag_of = [LAG] * 56 + [3, 3, 2, 2, 1, 1, 1, 1]
        sched = {}
        for a in range(NH * NCHUNK):
            sched.setdefault(a + lag_of[a], []).append(a)
        pending = []
        for g in range(NH * NCHUNK + 1):
            s, kc = divmod(g, NCHUNK)
            if s < NH:
                emit_energy(s, kc)
            if s == 0:
                emit_vproj(kc)
            elif pending:
                pending.pop(0)()
            if g == NCHUNK:
                # second head-pair projections, spread across stage 1
                for ns in range(4):
                    pending.append(
                        lambda ns=ns: emit_proj_chunk(wq_sb, xq_sb, qnd, qdp, 1, ns, ps_misc))
                for ns in range(4):
                    pending.append(
                        lambda ns=ns: emit_proj_chunk(wk_sb, xk_sb, knd, kdp, 1, ns, ps_misc))
            for a in sched.get(g, []):
                sa, kca = divmod(a, NCHUNK)
                emit_attv(sa, kca)
                if kca == NCHUNK - 1:
                    emit_normalize(sa)
                    if sa == 1:
                        # heads 0,1 done: transpose pair 01, spread out
                        for m in range(NCHUNK):
                            pending.append(lambda m=m: emit_transpose(0, m, ps_misc))

        # ---- tail: pair 23 transpose + out-proj + fp16 output DMA ----
        for fn in pending:
            fn()
        for m in range(NCHUNK):
            emit_transpose(1, m, ps_misc)
            emit_outproj(m)

    nc.compile()
    _built = nc
    return nc


def _host_prep(query, key, value, Wq, Wk, Wv, Wo, c):
    b, g = c // 2, c % 2
    DT = np.float16
    wqtd = np.empty((E, 256), np.float32)
    wktd = np.empty((E, 256), np.float32)
    wvt = np.empty((E, NH * D), np.float32)
    wot = np.empty((NH * D, E), np.float32)
    for i in range(NH):
        h = NH * g + i
        wqtd[:, D * i:D * (i + 1)] = Wq[D * h:D * (h + 1), :].T
        wktd[:, D * i:D * (i + 1)] = Wk[D * h:D * (h + 1), :].T
        wvt[:, D * i:D * (i + 1)] = Wv[D * h:D * (h + 1), :].T
        wot[D * i:D * (i + 1), :] = Wo[:, D * h:D * (h + 1)].T
    return {
        "xqT": np.ascontiguousarray(query[b].T).astype(DT),
        "xkT": np.ascontiguousarray(key[b].T).astype(DT),
        "xvT": np.ascontiguousarray(value[b].T).astype(DT),
        "wqtd": wqtd.astype(DT),
        "wktd": wktd.astype(DT),
        "wvt": wvt.astype(DT),
        "wot": wot.astype(DT),
        "iden": np.eye(128, dtype=DT),
    }


# test.py can flip these to profile
TRACE = False
TRACE_KWARGS = {}
LAST_RESULTS = None


def kernel(query, key, value, Wq, Wk, Wv, Wo):
    global LAST_RESULTS
    from concourse.bass_utils import run_bass_kernel_spmd

    args = [np.asarray(x, dtype=np.float32) for x in (query, key, value, Wq, Wk, Wv, Wo)]
    nc = _build()
    in_maps = [_host_prep(*args, c) for c in range(N_CORES)]
    res = run_bass_kernel_spmd(
        nc, in_maps, core_ids=list(range(N_CORES)), trace=TRACE, **TRACE_KWARGS
    )
    LAST_RESULTS = res
    outp = np.zeros((B, N, E), np.float32)
    for c in range(N_CORES):
        outp[c // 2] += res.results[c]["out"].astype(np.float32)
    return outp
